# revision 30
# baseline (speedup 1.0000x reference)
"""Trainium2 Bass kernel for nn_DilatedSpatioTemporalGCN.

Key observation: the reference masks adjacency with (softmax(...) > 0), and a
softmax output is strictly positive for these input magnitudes, so both
normalized adjacencies collapse to the constant matrix (J + I) / 513. The
static_MTE_matrix and dynamic-score einsum therefore do not affect the output
at all and are never read. What remains per batch element:

  per layer l (M_l = Ws[l] @ Wg[:F] + Wd[l] @ Wg[F:], folded through the gate):
    Z = c2 * (X + 1 colsum(X)^T) @ M_l + bias_row   (c2 = 1/513 with fp32 rounding)
    g = sigmoid(Z)
    r_l = relu(causal dilated conv_t(g) + bc[l]);  X_{l+1} = X + sum_j r_j
  then a tiny attention over the three r_l[t=T-1] slices.

Layout on-chip: partition p = (3 - t mod 4) * 32 + f, free = (t div 4, n);
the reversed group order puts t = T-1 at partition base 0. All feature
transforms are 128x128 block-structured fp32r matmuls; conv time shifts are
free-dim slot offsets plus partition-block shifts baked into the weights.

v2 structure (vs the v1 baseline):
- The rank-1 "+ 1 colsum(X)^T" bias term rides in column N of each activation
  tile: tiny accumulating matmuls against that column (plus a stacked
  bias-row lhsT and a one-hot rhs from the identity slab) produce the full
  sigmoid bias in PSUM, which the Activation op reads directly. No sx/racc
  bookkeeping, no DVE ops on the sigmoid critical path.
- Relu for layers 0/1 runs on Pool (tensor_scalar add+max) with accum_out
  writing the r colsum straight into column N. ACT only does sigmoids, the
  three attn tanhs and the final exp.
- Layer 2 is computed only where it is consumed: t in {3, 7, 11} (partition
  rows 0:32), so its Z matmuls, sigmoids, conv taps and relu are all
  32-partition ops with raw 32x32 weight blocks.
- res_agg^T is produced by PE transposes (identity rhs) of the already
  relu'd r[T-1] slices; the attention sv vector is produced transposed
  directly by per-chunk [128,1] matmuls against v. No staging or regroup
  DMAs anywhere in the tail; softmax + mix read PSUM directly.
- A dummy Exp right after the last sigmoid pulls the exp-table load off the
  tail critical path; a post-pass prunes the redundant table load that the
  bacc fixpoint hoists to function entry.

Sharding: data-parallel over batch (4 elements) on cores 0-3; cores 4-7
run duplicate batches (harmless, keeps all 8 cores uniform).
"""

import os
import numpy as np
from contextlib import ExitStack

import concourse.bacc as bacc
import concourse.tile as tile
from concourse import mybir
from concourse.bass_utils import run_bass_kernel_spmd

F32 = mybir.dt.float32
F32R = mybir.dt.float32r
AF = mybir.ActivationFunctionType
ALU = mybir.AluOpType
AX = mybir.AxisListType

BSZ, T, FD, N = 4, 12, 32, 512
L, K = 3, 3
DIL = (1, 2, 4)
NTHI = 3          # t div 4 chunks
NCHUNK = 4        # n chunks of 128

# logical time t lives at partition group 3 - (t % 4), free chunk t // 4.
# Host sends timesteps permuted so the natural DMA layout lands like that.
T_PERM = [4 * (t // 4) + 3 - (t % 4) for t in range(T)]

# wr column layout (ordered so the DMA priority line matches data needs:
# the small head slab carries everything the first bias matmuls touch)
ZW0 = 0                      # 128: block-diag Mc l0
IDN = ZW0 + 128              # 32: identity (rows 0:32)
BRT = IDN + 32               # 128: rows 0:3 = tiled bias_row per layer
WA = BRT + 128               # 32: Wa (rows 0:32)
V = WA + 32                  # 2: v padded to 2 cols (rows 0:32)
CW0 = V + 2                  # 5 x 128: conv l0 lhsT blocks
ZW1 = CW0 + 5 * 128          # 128: block-diag Mc l1
ZW2 = ZW1 + 128              # 32: raw Mc l2 (rows 0:32)
CW1 = ZW2 + 32               # 4 x 128: conv l1 lhsT blocks
CW2 = CW1 + 4 * 128          # 3 x 32: raw conv l2 blocks (rows 0:32)
WR_COLS = CW2 + 3 * 32

# wf columns: 0-2 tiled bc per layer, 3 ba (rows 0:32), 4-6 tiled bias_row
WF_COLS = 8

_CACHE = {}


def _conv_plan(l):
    """Per layer: list of (k, carry, {q_out: q_in}) matmul groups in the
    reversed-group layout."""
    d = DIL[l]
    plan = []
    for k in range(K):
        delta = (K - 1 - k) * d
        groups = {}
        for q_out in range(4):
            a = 3 - q_out                  # tau % 4
            ap_ = (a - delta) % 4
            carry = (a - delta - ap_) // 4
            groups.setdefault(carry, {})[q_out] = 3 - ap_
        for carry in sorted(groups):
            plan.append((k, carry, groups[carry]))
    return plan


def _prune_redundant_act_loads(nc):
    """Drop LoadActFuncSet instructions that are immediately superseded by
    another load on the ACT queue with no activation in between."""
    for blk in nc.main_func.blocks:
        insts = blk.instructions
        dead = []
        prev_load = None
        for inst in insts:
            if isinstance(inst, mybir.InstLoadActFuncSet):
                if prev_load is not None:
                    dead.append(prev_load)
                prev_load = inst
            elif isinstance(inst, mybir.InstActivation):
                prev_load = None
        for inst in dead:
            si = getattr(inst, "sync_info", None)
            if si is not None and (len(si.on_wait) > 0 or len(si.on_update) > 0):
                continue  # carries sync edges; not safe to drop
            insts.remove(inst)


def _build_nc():
    nc = bacc.Bacc("TRN2", target_bir_lowering=False, debug=False)
    emb_d = nc.dram_tensor("emb", [T, FD, N], F32R, kind="ExternalInput").ap()
    wr_d = nc.dram_tensor("wr", [128, WR_COLS], F32R, kind="ExternalInput").ap()
    wf_d = nc.dram_tensor("wf", [128, WF_COLS], F32, kind="ExternalInput").ap()
    y_d = nc.dram_tensor("y", [N, FD], F32, kind="ExternalOutput").ap()
    debug = bool(int(os.environ.get("K_DEBUG", "0")))
    if debug:
        dbg_g00 = nc.dram_tensor("dbg_g00", [128, N], F32, kind="ExternalOutput").ap()
        dbg_r00 = nc.dram_tensor("dbg_r00", [128, N], F32, kind="ExternalOutput").ap()
        dbg_bv0 = nc.dram_tensor("dbg_bv0", [128, NTHI], F32, kind="ExternalOutput").ap()
        dbg_scat = nc.dram_tensor("dbg_scat", [32, L * N], F32, kind="ExternalOutput").ap()
        dbg_esb = nc.dram_tensor("dbg_esb", [128, NCHUNK * 6], F32, kind="ExternalOutput").ap()
        dbg_prt = nc.dram_tensor("dbg_prt", [128, NCHUNK * L * FD], F32, kind="ExternalOutput").ap()
        dbg_g2 = nc.dram_tensor("dbg_g2", [32, N], F32, kind="ExternalOutput").ap()
        dbg_rs2 = nc.dram_tensor("dbg_rs2", [32, N], F32, kind="ExternalOutput").ap()

    trace_sim = bool(int(os.environ.get("K_TRACE_SIM", "0")))
    with tile.TileContext(nc, trace_sim=trace_sim) as tc, ExitStack() as ctx:
        ctx.enter_context(nc.allow_low_precision(
            "fp32r tiles feed fp32r matmuls; accumulation stays fp32 in PSUM"))
        wpool = ctx.enter_context(tc.tile_pool(name="w", bufs=1))
        spool = ctx.enter_context(tc.tile_pool(name="s", bufs=1))
        ppool_z = ctx.enter_context(tc.tile_pool(name="pz", bufs=3, space="PSUM"))
        ppool_c = ctx.enter_context(tc.tile_pool(name="pc", bufs=3, space="PSUM"))
        ppool_s = ctx.enter_context(tc.tile_pool(name="psm", bufs=2, space="PSUM"))
        ppool_k = ctx.enter_context(tc.tile_pool(name="pk", bufs=1, space="PSUM"))

        # ---- loads: interleave three HWDGE queues so the serialized issue
        # line and the DMA-engine transfer line both match data-need order ----
        embs = [wpool.tile([128, N + 2], F32R, name=f"emb{c}") for c in range(NTHI)]
        wr = wpool.tile([128, WR_COLS], F32R)
        wf = wpool.tile([128, WF_COLS], F32)
        emb_view = emb_d.rearrange("(thi tlo) f n -> (tlo f) thi n", tlo=4)

        # ---- ACT table warmup FIRST: the set-2 load must head the ACT queue
        # (any scalar-queue DMA issue emitted before it would delay it by the
        # serialized HWDGE line). PE p-state warmup likewise.
        dumin = spool.tile([128, 2], F32)
        nc.vector.memset(dumin[:], 0.0)
        duout = spool.tile([128, 2], F32)
        nc.scalar.activation(duout[:], dumin[:], AF.Sigmoid)
        dumm = spool.tile([128, 4], F32R)
        nc.vector.memset(dumm[:].bitcast(F32), 0.0)
        pwarm = ppool_s.tile([2, 2], F32, tag="sm", name="pwarm", bufs=1)
        nc.tensor.matmul(pwarm[:], dumm[:, 0:2], dumm[:, 2:4])

        # the shared HWDGE issue line serializes at ~630ns per DMA and the
        # DMA engines drain transfers in issue order, so alternate the two
        # HWDGE queues in data-need order
        nc.sync.dma_start(out=embs[0][:, 0:N], in_=emb_view[:, 0, :])
        nc.sync.dma_start(out=wr[:, ZW0:CW0], in_=wr_d[:, ZW0:CW0])
        nc.gpsimd.dma_start(out=wf, in_=wf_d)
        nc.sync.dma_start(out=embs[1][:, 0:N], in_=emb_view[:, 1, :])
        nc.sync.dma_start(out=embs[2][:, 0:N], in_=emb_view[:, 2, :])
        nc.sync.dma_start(out=wr[:, CW0:CW0 + 640], in_=wr_d[:, CW0:CW0 + 640])
        nc.sync.dma_start(out=wr[:, ZW1:ZW1 + 160], in_=wr_d[:, ZW1:ZW1 + 160])
        nc.sync.dma_start(out=wr[:, CW1:WR_COLS], in_=wr_d[:, CW1:WR_COLS])

        # ---- persistent tiles ----
        gps = [[wpool.tile([128, N], F32R, name=f"g{l}{c}") for c in range(NTHI)]
               for l in range(2)]
        bvs = [spool.tile([128, NTHI], F32, name=f"bv{l}") for l in range(2)]
        bv2 = spool.tile([32, NTHI], F32, name="bv2")
        g2 = [wpool.tile([32, N], F32R, name=f"g2{c}") for c in range(NTHI)]
        rs = [[wpool.tile([128, N + 2], F32R, name=f"r{l}{c}") for c in range(NTHI)]
              for l in range(2)]
        rs2 = wpool.tile([32, N], F32R, name="rs2")
        zeros = spool.tile([128, N], F32, name="zeros")
        nc.gpsimd.memset(zeros[:], 0.0)
        for c in range(NTHI):
            nc.gpsimd.memset(embs[c][:, N + 1:N + 2].bitcast(F32), 0.0)
        for l in range(2):
            for c in range(NTHI):
                nc.gpsimd.memset(rs[l][c][:, N + 1:N + 2].bitcast(F32), 0.0)
        s_cat = spool.tile([32, L * N], F32R, name="s_cat")
        e_sb = spool.tile([128, NCHUNK, 6], F32)
        es = spool.tile([128, NCHUNK], F32)
        ri = spool.tile([128, NCHUNK], F32)
        y_sb = spool.tile([128, NCHUNK, FD], F32)
        tmps = [spool.tile([128, FD], F32, name=f"ytmp{i}") for i in range(3)]
        pkt = ppool_k.tile([128, NCHUNK * L * FD + NCHUNK * 6], F32,
                           tag="prt", name="pkt")
        prt = pkt[:, 0:NCHUNK * L * FD].rearrange(
            "p (c l f) -> p c l f", c=NCHUNK, l=L)
        prt_sb = spool.tile([128, NCHUNK, L, FD], F32, name="prt_sb")
        psvT = pkt[:, NCHUNK * L * FD:].rearrange(
            "p (c s) -> p c s", c=NCHUNK)

        def svt_chunk(c):
            """The reference reshapes the layer-major sv concat [L*N] to
            [N, L], so node n's softmax triple is flat[3n:3n+3]. Chunk c
            covers flat[384c:384c+384]; lane j comes from a stride-3 lhsT
            window of the contiguous tanh concat (layer crossings land
            mid-window, which the contiguous layout absorbs)."""
            win = s_cat[0:32, 384 * c:384 * (c + 1)].rearrange(
                "p (i r) -> p r i", r=3)
            for j in range(3):
                nc.tensor.matmul(psvT[:, c, 2 * j:2 * j + 2],
                                 win[:, j, :], wr[0:32, V:V + 2])

        def attn(l, rsl2):
            """s = tanh(Wa^T r_l[T-1] + ba); sv^T and res_agg^T via PE."""
            ps = ppool_z.tile([32, N], F32, tag="zb", name=f"ps{l}")
            nc.tensor.matmul(ps[:], wr[0:32, WA:WA + 32], rsl2[0:32, 0:N])
            nc.scalar.activation(s_cat[:, l * N:(l + 1) * N], ps[:], AF.Tanh,
                                 bias=wf[0:32, 3:4], scale=1.0)
            for c in range(NCHUNK):
                nc.tensor.matmul(prt[:, c, l, :],
                                 rsl2[0:32, 128 * c:128 * (c + 1)],
                                 wr[0:32, IDN:IDN + 32])
            nc.vector.tensor_copy(prt_sb[:, :, l, :], prt[:, :, l, :])
            if l == 0:
                svt_chunk(0)
            elif l == 1:
                svt_chunk(1)
            else:
                svt_chunk(2)
                svt_chunk(3)

        # ================= layers 0 and 1 (full width) =================
        # sigmoid bias: colsum_n(Z) = Zw^T colsum(X) falls out of a single
        # free-dim reduce of the assembled pz, plus the constant bias row.
        zw_off = (ZW0, ZW1)
        cw_off = (CW0, CW1)
        for l in range(2):
            zw = wr[:, zw_off[l]:zw_off[l] + 128]
            psz = ppool_s.tile([128, 2 * NTHI], F32, tag="sm", name=f"psz{l}",
                               bufs=1)
            pzs = []
            for c in range(NTHI):
                pz = ppool_z.tile([128, N], F32, tag="zb", name=f"pz{l}{c}")
                nc.tensor.matmul(pz[:], zw, embs[c][:, 0:N],
                                 start=True, stop=True)
                if l == 0:
                    nc.vector.reduce_sum(embs[c][:, N:N + 1],
                                         embs[c][:, 0:N].bitcast(F32),
                                         axis=AX.X)
                nc.tensor.matmul(psz[:, 2 * c:2 * c + 2], zw,
                                 embs[c][:, N:N + 2], start=True, stop=False)
                nc.tensor.matmul(psz[:, 2 * c:2 * c + 2],
                                 wr[0:3, BRT:BRT + 128],
                                 wr[0:3, IDN + l:IDN + l + 2],
                                 start=False, stop=True)
                nc.vector.tensor_copy(bvs[l][:, c:c + 1],
                                      psz[:, 2 * c:2 * c + 1])
                pzs.append(pz)
            for c in range(NTHI):
                nc.scalar.activation(gps[l][c][:], pzs[c][:], AF.Sigmoid,
                                     bias=bvs[l][:, c:c + 1], scale=1.0)

            plan = _conv_plan(l)
            for c in range(NTHI):
                mms = [(i, k, carry) for i, (k, carry, _) in enumerate(plan)
                       if c + carry >= 0]
                pc = ppool_c.tile([128, N], F32, tag="cv", name=f"pc{l}{c}")
                for j, (i, k, carry) in enumerate(mms):
                    nc.tensor.matmul(
                        pc[:],
                        wr[:, cw_off[l] + i * 128: cw_off[l] + (i + 1) * 128],
                        gps[l][c + carry][:],
                        start=(j == 0), stop=(j == len(mms) - 1))
                if c == 1:
                    nc.scalar.activation(rs[l][c][:, 0:N], pc[:], AF.Relu,
                                         bias=wf[:, l:l + 1], scale=1.0,
                                         accum_out=rs[l][c][:, N:N + 1])
                else:
                    nc.vector.scalar_tensor_tensor(
                        rs[l][c][:, 0:N], pc[:], wf[:, l:l + 1], zeros[:],
                        ALU.add, ALU.max,
                        accum_out=rs[l][c][:, N:N + 1])
                rows = 128 if l == 0 else 32
                nc.gpsimd.tensor_tensor(embs[c][0:rows, 0:N + 1],
                                        embs[c][0:rows, 0:N + 1],
                                        rs[l][c][0:rows, 0:N + 1], ALU.add)
            if l == 1:
                attn(0, rs[0][2])

        # ================= layer 2 (rows 0:32 only: t in {3,7,11}) =========
        zw2 = wr[0:32, ZW2:ZW2 + 32]
        psz2 = ppool_s.tile([32, 2 * NTHI], F32, tag="sm", name="psz2", bufs=1)
        pz2s = []
        for c in range(NTHI):
            pz2 = ppool_z.tile([32, N], F32, tag="zb", name=f"pz2{c}")
            nc.tensor.matmul(pz2[:], zw2, embs[c][0:32, 0:N],
                             start=True, stop=True)
            nc.tensor.matmul(psz2[:, 2 * c:2 * c + 2], zw2,
                             embs[c][0:32, N:N + 2], start=True, stop=False)
            nc.tensor.matmul(psz2[:, 2 * c:2 * c + 2], wr[0:3, BRT:BRT + 32],
                             wr[0:3, IDN + 2:IDN + 4], start=False, stop=True)
            nc.vector.tensor_copy(bv2[:, c:c + 1], psz2[:, 2 * c:2 * c + 1])
            pz2s.append(pz2)
        for c in range(NTHI):
            nc.scalar.activation(g2[c][:], pz2s[c][:], AF.Sigmoid,
                                 bias=bv2[:, c:c + 1], scale=1.0)
        attn(1, rs[1][2])

        pc2 = ppool_c.tile([32, N], F32, tag="cv", name="pc2")
        for k in range(K):
            nc.tensor.matmul(pc2[:], wr[0:32, CW2 + 32 * k:CW2 + 32 * (k + 1)],
                             g2[k][:], start=(k == 0), stop=(k == K - 1))
        nc.vector.tensor_scalar(rs2[:], pc2[:], wf[0:32, 2:3], 0.0,
                                ALU.add, ALU.max)
        attn(2, rs2)

        # ================= softmax over L + weighted mix =================
        # exp via tanh half-angle (exact: e^x = (1+t)/(1-t), t = tanh(x/2));
        # the logits are O(1) so tanh never saturates, and it avoids loading
        # a second activation-table set for Exp. Odd psvT lanes hold 0, so
        # they contribute exactly 1.0 to the row sum; subtract 3 before the
        # reciprocal.
        u_sb = spool.tile([128, NCHUNK, 6], F32, name="u_sb")
        un = spool.tile([128, NCHUNK, 6], F32, name="un")
        nc.scalar.activation(u_sb[:], psvT[:], AF.Tanh, scale=0.5)
        nc.vector.tensor_scalar(un[:], u_sb[:], -1.0, 1.0,
                                ALU.mult, ALU.add)      # un = 1 - u
        nc.vector.reciprocal(un[:], un[:])
        nc.vector.tensor_scalar(u_sb[:], u_sb[:], 1.0, None, ALU.add)
        nc.vector.tensor_tensor(e_sb[:], u_sb[:], un[:], ALU.mult)
        nc.vector.tensor_reduce(es[:], e_sb[:], axis=AX.X, op=ALU.add)
        nc.vector.tensor_scalar(es[:], es[:], -3.0, None, ALU.add)
        nc.vector.reciprocal(ri[:], es[:])
        for c in (0, 1, 2, 3):
            if c != 2:
                eng, ta = nc.vector, tmps[0]
                eng.tensor_scalar(ta[:], prt_sb[:, c, 0, :], e_sb[:, c, 0:1],
                                  None, ALU.mult)
                eng.scalar_tensor_tensor(ta[:], prt_sb[:, c, 1, :],
                                         e_sb[:, c, 2:3], ta[:],
                                         ALU.mult, ALU.add)
                eng.scalar_tensor_tensor(ta[:], prt_sb[:, c, 2, :],
                                         e_sb[:, c, 4:5], ta[:],
                                         ALU.mult, ALU.add)
            else:
                # gpsimd has no scalar_tensor_tensor opcode
                eng, ta, tb = nc.gpsimd, tmps[1], tmps[2]
                eng.tensor_scalar(ta[:], prt_sb[:, c, 0, :], e_sb[:, c, 0:1],
                                  None, ALU.mult)
                eng.tensor_scalar(tb[:], prt_sb[:, c, 1, :], e_sb[:, c, 2:3],
                                  None, ALU.mult)
                eng.tensor_tensor(ta[:], ta[:], tb[:], ALU.add)
                eng.tensor_scalar(tb[:], prt_sb[:, c, 2, :], e_sb[:, c, 4:5],
                                  None, ALU.mult)
                eng.tensor_tensor(ta[:], ta[:], tb[:], ALU.add)
            eng.tensor_scalar(y_sb[:, c, :], ta[:], ri[:, c:c + 1], None,
                              ALU.mult)
        y_view = y_d.rearrange("(c p) f -> p c f", p=128)
        nc.sync.dma_start(out=y_view[:, 0:2, :], in_=y_sb[:, 0:2, :])
        nc.scalar.dma_start(out=y_view[:, 2:4, :], in_=y_sb[:, 2:4, :])
        if debug:
            nc.gpsimd.dma_start(out=dbg_g00, in_=gps[0][0][:].bitcast(F32))
            nc.gpsimd.dma_start(out=dbg_r00, in_=rs[0][0][:, 0:N].bitcast(F32))
            nc.gpsimd.dma_start(out=dbg_bv0, in_=bvs[0][:])
            nc.gpsimd.dma_start(out=dbg_scat, in_=s_cat[:].bitcast(F32))
            nc.gpsimd.dma_start(out=dbg_esb, in_=e_sb[:].rearrange("p a b -> p (a b)"))
            nc.gpsimd.dma_start(out=dbg_prt, in_=prt_sb[:].rearrange("p a b c -> p (a b c)"))
            nc.gpsimd.dma_start(out=dbg_g2, in_=g2[2][:].bitcast(F32))
            nc.gpsimd.dma_start(out=dbg_rs2, in_=rs2[:].bitcast(F32))

    nc.finalize()
    _prune_redundant_act_loads(nc)
    return nc


def _host_weights(Wd, bd, Ws, bs, Wg, bg, Wc, bc, Wa, ba, v):
    f32 = np.float32
    dinv = f32(1.0) / np.sqrt(f32(513.0))
    c2 = f32(dinv * dinv)

    wr = np.zeros((128, WR_COLS), np.float32)
    wf = np.zeros((128, WF_COLS), np.float32)

    Mcs, brs = [], []
    for l in range(L):
        M = (Ws[l] @ Wg[:FD] + Wd[l] @ Wg[FD:]).astype(f32)
        Mcs.append((c2 * M).astype(f32))
        brs.append((bs[l] @ Wg[:FD] + bd[l] @ Wg[FD:] + bg).astype(f32))
        wf[:, l] = np.tile(bc[l].astype(f32), 4)
        wf[:, 4 + l] = np.tile(brs[l], 4)
        wr[l:l + 1, BRT:BRT + 128] = np.tile(brs[l], 4)

    for l, off in ((0, ZW0), (1, ZW1)):
        for q in range(4):
            wr[32 * q:32 * (q + 1), off + 32 * q: off + 32 * (q + 1)] = Mcs[l]
    wr[0:32, ZW2:ZW2 + 32] = Mcs[2]

    for l, cw in ((0, CW0), (1, CW1)):
        plan = _conv_plan(l)
        for i, (k, carry, groups) in enumerate(plan):
            blk = Wc[l][:, :, 0, k].T.astype(f32)   # [fi, fo]
            for q_out, q_in in groups.items():
                wr[32 * q_in:32 * (q_in + 1),
                   cw + i * 128 + 32 * q_out:
                   cw + i * 128 + 32 * (q_out + 1)] = blk
    for k in range(K):
        wr[0:32, CW2 + 32 * k:CW2 + 32 * (k + 1)] = Wc[2][:, :, 0, k].T

    wr[0:32, IDN:IDN + 32] = np.eye(32, dtype=f32)
    wr[0:32, WA:WA + 32] = Wa.astype(f32)
    wr[0:32, V:V + 1] = v.astype(f32)
    wf[0:32, 3] = ba.astype(f32)
    return wr, wf


def kernel(**inputs):
    node_embeddings = np.asarray(inputs["node_embeddings"], dtype=np.float32)
    wr, wf = _host_weights(
        np.asarray(inputs["Wd"], np.float32), np.asarray(inputs["bd"], np.float32),
        np.asarray(inputs["Ws"], np.float32), np.asarray(inputs["bs"], np.float32),
        np.asarray(inputs["Wg"], np.float32), np.asarray(inputs["bg"], np.float32),
        np.asarray(inputs["Wc"], np.float32), np.asarray(inputs["bc"], np.float32),
        np.asarray(inputs["Wa"], np.float32), np.asarray(inputs["ba"], np.float32),
        np.asarray(inputs["v"], np.float32),
    )

    if "nc" not in _CACHE:
        _CACHE["nc"] = _build_nc()
    nc = _CACHE["nc"]

    n_cores = 8
    in_maps = [
        {"emb": np.ascontiguousarray(node_embeddings[i % BSZ][T_PERM]),
         "wr": wr, "wf": wf}
        for i in range(n_cores)
    ]
    res = run_bass_kernel_spmd(nc, in_maps, core_ids=list(range(n_cores)))
    y = np.stack([res.results[b]["y"] for b in range(BSZ)], axis=0)
    return y.astype(np.float32)


# revision 31
# speedup vs baseline: 1.0579x; 1.0579x over previous
"""Trainium2 Bass kernel for nn_DilatedSpatioTemporalGCN.

Key observation: the reference masks adjacency with (softmax(...) > 0), and a
softmax output is strictly positive for these input magnitudes, so both
normalized adjacencies collapse to the constant matrix (J + I) / 513. The
static_MTE_matrix and dynamic-score einsum therefore do not affect the output
at all and are never read. What remains per batch element:

  per layer l (M_l = Ws[l] @ Wg[:F] + Wd[l] @ Wg[F:], folded through the gate):
    Z = c2 * (X + 1 colsum(X)^T) @ M_l + bias_row   (c2 = 1/513 with fp32 rounding)
    g = sigmoid(Z)
    r_l = relu(causal dilated conv_t(g) + bc[l]);  X_{l+1} = X + sum_j r_j
  then a tiny attention over the three r_l[t=T-1] slices.

Layout on-chip: partition p = (3 - t mod 4) * 32 + f, free = (t div 4, n);
the reversed group order puts t = T-1 at partition base 0. All feature
transforms are 128x128 block-structured fp32r matmuls; conv time shifts are
free-dim slot offsets plus partition-block shifts baked into the weights.

v2 structure (vs the v1 baseline):
- The rank-1 "+ 1 colsum(X)^T" bias term rides in column N of each activation
  tile: tiny accumulating matmuls against that column (plus a stacked
  bias-row lhsT and a one-hot rhs from the identity slab) produce the full
  sigmoid bias in PSUM, which the Activation op reads directly. No sx/racc
  bookkeeping, no DVE ops on the sigmoid critical path.
- Relu for layers 0/1 runs on Pool (tensor_scalar add+max) with accum_out
  writing the r colsum straight into column N. ACT only does sigmoids, the
  three attn tanhs and the final exp.
- Layer 2 is computed only where it is consumed: t in {3, 7, 11} (partition
  rows 0:32), so its Z matmuls, sigmoids, conv taps and relu are all
  32-partition ops with raw 32x32 weight blocks.
- res_agg^T is produced by PE transposes (identity rhs) of the already
  relu'd r[T-1] slices; the attention sv vector is produced transposed
  directly by per-chunk [128,1] matmuls against v. No staging or regroup
  DMAs anywhere in the tail; softmax + mix read PSUM directly.
- A dummy Exp right after the last sigmoid pulls the exp-table load off the
  tail critical path; a post-pass prunes the redundant table load that the
  bacc fixpoint hoists to function entry.

Sharding: data-parallel over batch (4 elements) on cores 0-3; cores 4-7
run duplicate batches (harmless, keeps all 8 cores uniform).
"""

import os
import numpy as np
from contextlib import ExitStack

import concourse.bacc as bacc
import concourse.tile as tile
from concourse import mybir
from concourse.bass_utils import run_bass_kernel_spmd

F32 = mybir.dt.float32
F32R = mybir.dt.float32r
AF = mybir.ActivationFunctionType
ALU = mybir.AluOpType
AX = mybir.AxisListType

BSZ, T, FD, N = 4, 12, 32, 512
L, K = 3, 3
DIL = (1, 2, 4)
NTHI = 3          # t div 4 chunks
NCHUNK = 4        # n chunks of 128

# logical time t lives at partition group 3 - (t % 4), free chunk t // 4.
# Host sends timesteps permuted so the natural DMA layout lands like that.
T_PERM = [4 * (t // 4) + 3 - (t % 4) for t in range(T)]

# wr column layout (ordered so the DMA priority line matches data needs:
# the small head slab carries everything the first bias matmuls touch)
ZW0 = 0                      # 128: block-diag Mc l0
IDN = ZW0 + 128              # 32: identity (rows 0:32)
BRT = IDN + 32               # 128: rows 0:3 = tiled bias_row per layer
WA = BRT + 128               # 32: Wa (rows 0:32)
V = WA + 32                  # 2: v padded to 2 cols (rows 0:32)
CW0 = V + 2                  # 5 x 128: conv l0 lhsT blocks
ZW1 = CW0 + 5 * 128          # 128: block-diag Mc l1
ZW2 = ZW1 + 128              # 32: raw Mc l2 (rows 0:32)
CW1 = ZW2 + 32               # 4 x 128: conv l1 lhsT blocks
CW2 = CW1 + 4 * 128          # 3 x 32: raw conv l2 blocks (rows 0:32)
WR_COLS = CW2 + 3 * 32

# wf columns: 0-2 tiled bc per layer, 3 ba (rows 0:32), 4-6 tiled bias_row
WF_COLS = 8

_CACHE = {}


def _conv_plan(l):
    """Per layer: list of (k, carry, {q_out: q_in}) matmul groups in the
    reversed-group layout."""
    d = DIL[l]
    plan = []
    for k in range(K):
        delta = (K - 1 - k) * d
        groups = {}
        for q_out in range(4):
            a = 3 - q_out                  # tau % 4
            ap_ = (a - delta) % 4
            carry = (a - delta - ap_) // 4
            groups.setdefault(carry, {})[q_out] = 3 - ap_
        for carry in sorted(groups):
            plan.append((k, carry, groups[carry]))
    return plan


def _prune_redundant_act_loads(nc):
    """Drop LoadActFuncSet instructions that are immediately superseded by
    another load on the ACT queue with no activation in between."""
    for blk in nc.main_func.blocks:
        insts = blk.instructions
        dead = []
        prev_load = None
        for inst in insts:
            if isinstance(inst, mybir.InstLoadActFuncSet):
                if prev_load is not None:
                    dead.append(prev_load)
                prev_load = inst
            elif isinstance(inst, mybir.InstActivation):
                prev_load = None
        for inst in dead:
            si = getattr(inst, "sync_info", None)
            if si is not None and (len(si.on_wait) > 0 or len(si.on_update) > 0):
                continue  # carries sync edges; not safe to drop
            insts.remove(inst)


def _build_nc():
    nc = bacc.Bacc("TRN2", target_bir_lowering=False, debug=False)
    emb_d = nc.dram_tensor("emb", [T, FD, N], F32R, kind="ExternalInput").ap()
    wr_d = nc.dram_tensor("wr", [128, WR_COLS], F32R, kind="ExternalInput").ap()
    wf_d = nc.dram_tensor("wf", [128, WF_COLS], F32, kind="ExternalInput").ap()
    y_d = nc.dram_tensor("y", [N, FD], F32, kind="ExternalOutput").ap()
    debug = bool(int(os.environ.get("K_DEBUG", "0")))
    if debug:
        dbg_g00 = nc.dram_tensor("dbg_g00", [128, N], F32, kind="ExternalOutput").ap()
        dbg_r00 = nc.dram_tensor("dbg_r00", [128, N], F32, kind="ExternalOutput").ap()
        dbg_bv0 = nc.dram_tensor("dbg_bv0", [128, NTHI], F32, kind="ExternalOutput").ap()
        dbg_scat = nc.dram_tensor("dbg_scat", [32, L * N], F32, kind="ExternalOutput").ap()
        dbg_esb = nc.dram_tensor("dbg_esb", [128, NCHUNK * 6], F32, kind="ExternalOutput").ap()
        dbg_prt = nc.dram_tensor("dbg_prt", [128, NCHUNK * L * FD], F32, kind="ExternalOutput").ap()
        dbg_g2 = nc.dram_tensor("dbg_g2", [32, N], F32, kind="ExternalOutput").ap()
        dbg_rs2 = nc.dram_tensor("dbg_rs2", [32, N], F32, kind="ExternalOutput").ap()

    trace_sim = bool(int(os.environ.get("K_TRACE_SIM", "0")))
    with tile.TileContext(nc, trace_sim=trace_sim) as tc, ExitStack() as ctx:
        ctx.enter_context(nc.allow_low_precision(
            "fp32r tiles feed fp32r matmuls; accumulation stays fp32 in PSUM"))
        wpool = ctx.enter_context(tc.tile_pool(name="w", bufs=1))
        spool = ctx.enter_context(tc.tile_pool(name="s", bufs=1))
        ppool_z = ctx.enter_context(tc.tile_pool(name="pz", bufs=3, space="PSUM"))
        ppool_c = ctx.enter_context(tc.tile_pool(name="pc", bufs=3, space="PSUM"))
        ppool_s = ctx.enter_context(tc.tile_pool(name="psm", bufs=2, space="PSUM"))
        ppool_k = ctx.enter_context(tc.tile_pool(name="pk", bufs=1, space="PSUM"))

        # ---- loads: interleave three HWDGE queues so the serialized issue
        # line and the DMA-engine transfer line both match data-need order ----
        embs = [wpool.tile([128, N + 2], F32R, name=f"emb{c}") for c in range(NTHI)]
        wr = wpool.tile([128, WR_COLS], F32R)
        wf = wpool.tile([128, WF_COLS], F32)
        emb_view = emb_d.rearrange("(thi tlo) f n -> (tlo f) thi n", tlo=4)

        # ---- ACT table warmup FIRST: the set-2 load must head the ACT queue
        # (any scalar-queue DMA issue emitted before it would delay it by the
        # serialized HWDGE line). PE p-state warmup likewise.
        dumin = spool.tile([128, 2], F32)
        nc.vector.memset(dumin[:], 0.0)
        duout = spool.tile([128, 2], F32)
        nc.scalar.activation(duout[:], dumin[:], AF.Sigmoid)
        dumm = spool.tile([128, 4], F32R)
        nc.vector.memset(dumm[:].bitcast(F32), 0.0)
        pwarm = ppool_s.tile([2, 2], F32, tag="sm", name="pwarm", bufs=1)
        nc.tensor.matmul(pwarm[:], dumm[:, 0:2], dumm[:, 2:4])

        # the shared HWDGE issue line serializes at ~630ns per DMA and the
        # DMA engines drain transfers in issue order, so alternate the two
        # HWDGE queues in data-need order
        nc.sync.dma_start(out=embs[0][:, 0:N], in_=emb_view[:, 0, :])
        nc.sync.dma_start(out=wr[:, ZW0:CW0], in_=wr_d[:, ZW0:CW0])
        nc.gpsimd.dma_start(out=wf, in_=wf_d)
        nc.sync.dma_start(out=embs[1][:, 0:N], in_=emb_view[:, 1, :])
        nc.sync.dma_start(out=embs[2][:, 0:N], in_=emb_view[:, 2, :])
        nc.sync.dma_start(out=wr[:, CW0:CW0 + 640], in_=wr_d[:, CW0:CW0 + 640])
        nc.sync.dma_start(out=wr[:, ZW1:ZW1 + 160], in_=wr_d[:, ZW1:ZW1 + 160])
        nc.sync.dma_start(out=wr[:, CW1:WR_COLS], in_=wr_d[:, CW1:WR_COLS])

        # ---- persistent tiles ----
        gps = [[wpool.tile([128, N], F32R, name=f"g{l}{c}") for c in range(NTHI)]
               for l in range(2)]
        bvs = [spool.tile([128, NTHI], F32, name=f"bv{l}") for l in range(2)]
        bv2 = spool.tile([32, NTHI], F32, name="bv2")
        g2 = [wpool.tile([32, N], F32R, name=f"g2{c}") for c in range(NTHI)]
        rs = [[wpool.tile([128, N + 2], F32R, name=f"r{l}{c}") for c in range(NTHI)]
              for l in range(2)]
        rs2 = wpool.tile([32, N], F32R, name="rs2")
        zeros = spool.tile([128, N], F32, name="zeros")
        nc.gpsimd.memset(zeros[:], 0.0)
        for c in range(NTHI):
            nc.gpsimd.memset(embs[c][:, N + 1:N + 2].bitcast(F32), 0.0)
        for l in range(2):
            for c in range(NTHI):
                nc.gpsimd.memset(rs[l][c][:, N + 1:N + 2].bitcast(F32), 0.0)
        s_cat = spool.tile([32, L * N], F32R, name="s_cat")
        e_sb = spool.tile([128, NCHUNK, 6], F32)
        es = spool.tile([128, NCHUNK], F32)
        ri = spool.tile([128, NCHUNK], F32)
        y_sb = spool.tile([128, NCHUNK, FD], F32)
        tmps = [spool.tile([128, FD], F32, name=f"ytmp{i}") for i in range(3)]
        pkt = ppool_k.tile([128, NCHUNK * L * FD + NCHUNK * 6], F32,
                           tag="prt", name="pkt")
        prt = pkt[:, 0:NCHUNK * L * FD].rearrange(
            "p (c l f) -> p c l f", c=NCHUNK, l=L)
        prt_sb = spool.tile([128, NCHUNK, L, FD], F32, name="prt_sb")
        psvT = pkt[:, NCHUNK * L * FD:].rearrange(
            "p (c s) -> p c s", c=NCHUNK)

        def svt_chunk(c):
            """The reference reshapes the layer-major sv concat [L*N] to
            [N, L], so node n's softmax triple is flat[3n:3n+3]. Chunk c
            covers flat[384c:384c+384]; lane j comes from a stride-3 lhsT
            window of the contiguous tanh concat (layer crossings land
            mid-window, which the contiguous layout absorbs)."""
            win = s_cat[0:32, 384 * c:384 * (c + 1)].rearrange(
                "p (i r) -> p r i", r=3)
            for j in range(3):
                nc.tensor.matmul(psvT[:, c, 2 * j:2 * j + 2],
                                 win[:, j, :], wr[0:32, V:V + 2])

        def attn(l, rsl2):
            """s = tanh(Wa^T r_l[T-1] + ba); sv^T and res_agg^T via PE."""
            ps = ppool_z.tile([32, N], F32, tag="zb", name=f"ps{l}")
            nc.tensor.matmul(ps[:], wr[0:32, WA:WA + 32], rsl2[0:32, 0:N])
            nc.scalar.activation(s_cat[:, l * N:(l + 1) * N], ps[:], AF.Tanh,
                                 bias=wf[0:32, 3:4], scale=1.0)
            for c in range(NCHUNK):
                nc.tensor.matmul(prt[:, c, l, :],
                                 rsl2[0:32, 128 * c:128 * (c + 1)],
                                 wr[0:32, IDN:IDN + 32])
            nc.vector.tensor_copy(prt_sb[:, :, l, :], prt[:, :, l, :])
            if l == 0:
                svt_chunk(0)
            elif l == 1:
                svt_chunk(1)
            else:
                svt_chunk(2)
                svt_chunk(3)

        # ================= layers 0 and 1 (full width) =================
        # sigmoid bias: colsum_n(Z) = Zw^T colsum(X) falls out of a single
        # free-dim reduce of the assembled pz, plus the constant bias row.
        zw_off = (ZW0, ZW1)
        cw_off = (CW0, CW1)
        for l in range(2):
            zw = wr[:, zw_off[l]:zw_off[l] + 128]
            psz = ppool_s.tile([128, 2 * NTHI], F32, tag="sm", name=f"psz{l}",
                               bufs=1)
            pzs = []
            for c in range(NTHI):
                pz = ppool_z.tile([128, N], F32, tag="zb", name=f"pz{l}{c}")
                rhss = [embs[c]] + [rs[j][c] for j in range(l)]
                for i, rhs in enumerate(rhss):
                    nc.tensor.matmul(pz[:], zw, rhs[:, 0:N],
                                     start=(i == 0), stop=(i == len(rhss) - 1))
                if l == 0:
                    nc.vector.reduce_sum(embs[c][:, N:N + 1],
                                         embs[c][:, 0:N].bitcast(F32),
                                         axis=AX.X)
                for i, rhs in enumerate(rhss):
                    nc.tensor.matmul(psz[:, 2 * c:2 * c + 2], zw,
                                     rhs[:, N:N + 2],
                                     start=(i == 0), stop=False)
                nc.tensor.matmul(psz[:, 2 * c:2 * c + 2],
                                 wr[0:3, BRT:BRT + 128],
                                 wr[0:3, IDN + l:IDN + l + 2],
                                 start=False, stop=True)
                nc.vector.tensor_copy(bvs[l][:, c:c + 1],
                                      psz[:, 2 * c:2 * c + 1])
                pzs.append(pz)
            for c in range(NTHI):
                nc.scalar.activation(gps[l][c][:], pzs[c][:], AF.Sigmoid,
                                     bias=bvs[l][:, c:c + 1], scale=1.0)

            plan = _conv_plan(l)
            for c in range(NTHI):
                mms = [(i, k, carry) for i, (k, carry, _) in enumerate(plan)
                       if c + carry >= 0]
                pc = ppool_c.tile([128, N], F32, tag="cv", name=f"pc{l}{c}")
                for j, (i, k, carry) in enumerate(mms):
                    nc.tensor.matmul(
                        pc[:],
                        wr[:, cw_off[l] + i * 128: cw_off[l] + (i + 1) * 128],
                        gps[l][c + carry][:],
                        start=(j == 0), stop=(j == len(mms) - 1))
                if c == 1:
                    nc.scalar.activation(rs[l][c][:, 0:N], pc[:], AF.Relu,
                                         bias=wf[:, l:l + 1], scale=1.0,
                                         accum_out=rs[l][c][:, N:N + 1])
                else:
                    nc.vector.scalar_tensor_tensor(
                        rs[l][c][:, 0:N], pc[:], wf[:, l:l + 1], zeros[:],
                        ALU.add, ALU.max,
                        accum_out=rs[l][c][:, N:N + 1])
            if l == 1:
                attn(0, rs[0][2])

        # ================= layer 2 (rows 0:32 only: t in {3,7,11}) =========
        zw2 = wr[0:32, ZW2:ZW2 + 32]
        psz2 = ppool_s.tile([32, 2 * NTHI], F32, tag="sm", name="psz2", bufs=1)
        pz2s = []
        for c in range(NTHI):
            pz2 = ppool_z.tile([32, N], F32, tag="zb", name=f"pz2{c}")
            rhss = [embs[c], rs[0][c], rs[1][c]]
            for i, rhs in enumerate(rhss):
                nc.tensor.matmul(pz2[:], zw2, rhs[0:32, 0:N],
                                 start=(i == 0), stop=(i == len(rhss) - 1))
            for i, rhs in enumerate(rhss):
                nc.tensor.matmul(psz2[:, 2 * c:2 * c + 2], zw2,
                                 rhs[0:32, N:N + 2],
                                 start=(i == 0), stop=False)
            nc.tensor.matmul(psz2[:, 2 * c:2 * c + 2], wr[0:3, BRT:BRT + 32],
                             wr[0:3, IDN + 2:IDN + 4], start=False, stop=True)
            nc.vector.tensor_copy(bv2[:, c:c + 1], psz2[:, 2 * c:2 * c + 1])
            pz2s.append(pz2)
        for c in range(NTHI):
            nc.scalar.activation(g2[c][:], pz2s[c][:], AF.Sigmoid,
                                 bias=bv2[:, c:c + 1], scale=1.0)
        attn(1, rs[1][2])

        pc2 = ppool_c.tile([32, N], F32, tag="cv", name="pc2")
        for k in range(K):
            nc.tensor.matmul(pc2[:], wr[0:32, CW2 + 32 * k:CW2 + 32 * (k + 1)],
                             g2[k][:], start=(k == 0), stop=(k == K - 1))
        nc.vector.tensor_scalar(rs2[:], pc2[:], wf[0:32, 2:3], 0.0,
                                ALU.add, ALU.max)
        attn(2, rs2)

        # ================= softmax over L + weighted mix =================
        # exp via tanh half-angle (exact: e^x = (1+t)/(1-t), t = tanh(x/2));
        # the logits are O(1) so tanh never saturates, and it avoids loading
        # a second activation-table set for Exp. Odd psvT lanes hold 0, so
        # they contribute exactly 1.0 to the row sum; subtract 3 before the
        # reciprocal.
        u_sb = spool.tile([128, NCHUNK, 6], F32, name="u_sb")
        un = spool.tile([128, NCHUNK, 6], F32, name="un")
        nc.scalar.activation(u_sb[:], psvT[:], AF.Tanh, scale=0.5)
        nc.vector.tensor_scalar(un[:], u_sb[:], -1.0, 1.0,
                                ALU.mult, ALU.add)      # un = 1 - u
        nc.vector.reciprocal(un[:], un[:])
        nc.vector.tensor_scalar(u_sb[:], u_sb[:], 1.0, None, ALU.add)
        nc.vector.tensor_tensor(e_sb[:], u_sb[:], un[:], ALU.mult)
        nc.vector.tensor_reduce(es[:], e_sb[:], axis=AX.X, op=ALU.add)
        nc.vector.tensor_scalar(es[:], es[:], -3.0, None, ALU.add)
        nc.vector.reciprocal(ri[:], es[:])
        for c in (0, 1, 2, 3):
            if c != 2:
                eng, ta = nc.vector, tmps[0]
                eng.tensor_scalar(ta[:], prt_sb[:, c, 0, :], e_sb[:, c, 0:1],
                                  None, ALU.mult)
                eng.scalar_tensor_tensor(ta[:], prt_sb[:, c, 1, :],
                                         e_sb[:, c, 2:3], ta[:],
                                         ALU.mult, ALU.add)
                eng.scalar_tensor_tensor(ta[:], prt_sb[:, c, 2, :],
                                         e_sb[:, c, 4:5], ta[:],
                                         ALU.mult, ALU.add)
            else:
                # gpsimd has no scalar_tensor_tensor opcode
                eng, ta, tb = nc.gpsimd, tmps[1], tmps[2]
                eng.tensor_scalar(ta[:], prt_sb[:, c, 0, :], e_sb[:, c, 0:1],
                                  None, ALU.mult)
                eng.tensor_scalar(tb[:], prt_sb[:, c, 1, :], e_sb[:, c, 2:3],
                                  None, ALU.mult)
                eng.tensor_tensor(ta[:], ta[:], tb[:], ALU.add)
                eng.tensor_scalar(tb[:], prt_sb[:, c, 2, :], e_sb[:, c, 4:5],
                                  None, ALU.mult)
                eng.tensor_tensor(ta[:], ta[:], tb[:], ALU.add)
            eng.tensor_scalar(y_sb[:, c, :], ta[:], ri[:, c:c + 1], None,
                              ALU.mult)
        y_view = y_d.rearrange("(c p) f -> p c f", p=128)
        nc.sync.dma_start(out=y_view[:, 0:2, :], in_=y_sb[:, 0:2, :])
        nc.scalar.dma_start(out=y_view[:, 2:4, :], in_=y_sb[:, 2:4, :])
        if debug:
            nc.gpsimd.dma_start(out=dbg_g00, in_=gps[0][0][:].bitcast(F32))
            nc.gpsimd.dma_start(out=dbg_r00, in_=rs[0][0][:, 0:N].bitcast(F32))
            nc.gpsimd.dma_start(out=dbg_bv0, in_=bvs[0][:])
            nc.gpsimd.dma_start(out=dbg_scat, in_=s_cat[:].bitcast(F32))
            nc.gpsimd.dma_start(out=dbg_esb, in_=e_sb[:].rearrange("p a b -> p (a b)"))
            nc.gpsimd.dma_start(out=dbg_prt, in_=prt_sb[:].rearrange("p a b c -> p (a b c)"))
            nc.gpsimd.dma_start(out=dbg_g2, in_=g2[2][:].bitcast(F32))
            nc.gpsimd.dma_start(out=dbg_rs2, in_=rs2[:].bitcast(F32))

    nc.finalize()
    _prune_redundant_act_loads(nc)
    return nc


def _host_weights(Wd, bd, Ws, bs, Wg, bg, Wc, bc, Wa, ba, v):
    f32 = np.float32
    dinv = f32(1.0) / np.sqrt(f32(513.0))
    c2 = f32(dinv * dinv)

    wr = np.zeros((128, WR_COLS), np.float32)
    wf = np.zeros((128, WF_COLS), np.float32)

    Mcs, brs = [], []
    for l in range(L):
        M = (Ws[l] @ Wg[:FD] + Wd[l] @ Wg[FD:]).astype(f32)
        Mcs.append((c2 * M).astype(f32))
        brs.append((bs[l] @ Wg[:FD] + bd[l] @ Wg[FD:] + bg).astype(f32))
        wf[:, l] = np.tile(bc[l].astype(f32), 4)
        wf[:, 4 + l] = np.tile(brs[l], 4)
        wr[l:l + 1, BRT:BRT + 128] = np.tile(brs[l], 4)

    for l, off in ((0, ZW0), (1, ZW1)):
        for q in range(4):
            wr[32 * q:32 * (q + 1), off + 32 * q: off + 32 * (q + 1)] = Mcs[l]
    wr[0:32, ZW2:ZW2 + 32] = Mcs[2]

    for l, cw in ((0, CW0), (1, CW1)):
        plan = _conv_plan(l)
        for i, (k, carry, groups) in enumerate(plan):
            blk = Wc[l][:, :, 0, k].T.astype(f32)   # [fi, fo]
            for q_out, q_in in groups.items():
                wr[32 * q_in:32 * (q_in + 1),
                   cw + i * 128 + 32 * q_out:
                   cw + i * 128 + 32 * (q_out + 1)] = blk
    for k in range(K):
        wr[0:32, CW2 + 32 * k:CW2 + 32 * (k + 1)] = Wc[2][:, :, 0, k].T

    wr[0:32, IDN:IDN + 32] = np.eye(32, dtype=f32)
    wr[0:32, WA:WA + 32] = Wa.astype(f32)
    wr[0:32, V:V + 1] = v.astype(f32)
    wf[0:32, 3] = ba.astype(f32)
    return wr, wf


def kernel(**inputs):
    node_embeddings = np.asarray(inputs["node_embeddings"], dtype=np.float32)
    wr, wf = _host_weights(
        np.asarray(inputs["Wd"], np.float32), np.asarray(inputs["bd"], np.float32),
        np.asarray(inputs["Ws"], np.float32), np.asarray(inputs["bs"], np.float32),
        np.asarray(inputs["Wg"], np.float32), np.asarray(inputs["bg"], np.float32),
        np.asarray(inputs["Wc"], np.float32), np.asarray(inputs["bc"], np.float32),
        np.asarray(inputs["Wa"], np.float32), np.asarray(inputs["ba"], np.float32),
        np.asarray(inputs["v"], np.float32),
    )

    if "nc" not in _CACHE:
        _CACHE["nc"] = _build_nc()
    nc = _CACHE["nc"]

    n_cores = 8
    in_maps = [
        {"emb": np.ascontiguousarray(node_embeddings[i % BSZ][T_PERM]),
         "wr": wr, "wf": wf}
        for i in range(n_cores)
    ]
    res = run_bass_kernel_spmd(nc, in_maps, core_ids=list(range(n_cores)))
    y = np.stack([res.results[b]["y"] for b in range(BSZ)], axis=0)
    return y.astype(np.float32)
